# revision 6
# baseline (speedup 1.0000x reference)
"""ConvNearestNeightbor Trainium2 kernel.

out[b, n*C+c, i, j] = max_k |x[b,c,i-r_k,j-c_k] - neighbors[n,c,k]|
over the 9 zero-padded 3x3 shifts (r_k, c_k).

Sharding: 8 cores = 4 batch-groups x 2 num-groups.
Per core: B_loc=4 batches, N_loc=16 codebook entries.
Partition layout: (nn in 0..3, c in 0..31) -> 128 partitions, with the
codebook tile index nt in 0..3 selecting n = nt*4+nn.
Free dim: (b in 0..3, 32x32 pixels) = 4096.

Compute per (nt): 1 tensor_scalar(sub, abs_max, 0) to init acc = |x - nb_0|,
then 8 scalar_tensor_tensor(sub, abs_max) ops: acc = max(acc, |x_win - nb_k|).
x windows are strided reads of a zero-padded 34x34 image held in SBUF.
All f32 -> bit-exact vs the fp32 reference.
"""

import numpy as np

B, C, H, W = 16, 32, 32, 32
NUM = 32
NCORES = 8
BG, NG = 4, 2          # batch groups x num groups
B_LOC = B // BG        # 4
N_LOC = NUM // NG      # 16
NT = N_LOC // 4        # 4 codebook tiles of 4 n each
PH, PW = H + 2, W + 2  # 34 x 34 padded image
FREE = B_LOC * H * W   # 4096

_module_cache = {}


def _build_module():
    import concourse.bacc as bacc
    import concourse.mybir as mybir
    import concourse.tile as tile

    dt = mybir.dt
    Alu = mybir.AluOpType
    AF = mybir.ActivationFunctionType

    nc = bacc.Bacc("TRN2", debug=False)
    x = nc.dram_tensor("x", [B_LOC, C, H, W], dt.float32, kind="ExternalInput")
    nb = nc.dram_tensor("neighbors", [N_LOC, C, 9], dt.float32, kind="ExternalInput")
    out = nc.dram_tensor(
        "out", [B_LOC, N_LOC * C, H, W], dt.float32, kind="ExternalOutput"
    )

    # window start offsets within the padded 34x34 image for the 9 shifts
    # k = (row+1)*3 + (col+1), window starts at (1-row, 1-col)
    offs = []
    for row in (-1, 0, 1):
        for col in (-1, 0, 1):
            offs.append((1 - row, 1 - col))

    with tile.TileContext(nc) as tc:
        with (
            tc.tile_pool(name="const", bufs=1) as cpool,
            tc.tile_pool(name="accp", bufs=3) as apool,
            tc.tile_pool(name="dp", bufs=4) as dpool,
        ):
            nbt = cpool.tile([128, NT * 9], dt.float32, tag="nbt")
            # nbt[(nn,c), (t,k)] = neighbors[t*4+nn, c, k]
            nb_src = nb.ap().rearrange("(t nn) c k -> (nn c) t k", nn=4)
            nbt_v = nbt[:].rearrange("p (t k) -> p t k", t=NT)
            nc.sync.dma_start(nbt_v, nb_src)
            # negated neighbors: ACT bias computes Abs(x + (-nb))
            nbneg = cpool.tile([128, NT * 9], dt.float32, tag="nbneg")
            nc.vector.tensor_scalar(nbneg[:], nbt[:], -1.0, None, Alu.mult)

            xpad = cpool.tile([128, B_LOC * PH * PW], dt.float32, tag="xpad")
            nc.vector.memset(xpad[:], 0.0)
            xpad_v = xpad[:].rearrange("p (b h w) -> p b h w", b=B_LOC, h=PH, w=PW)
            # interior loads: partition group nn gets a full copy of x
            x_src = x.ap().rearrange("b c h w -> c b h w")
            for nn in range(4):
                for bb in range(B_LOC):
                    nc.sync.dma_start(
                        xpad_v[nn * 32 : (nn + 1) * 32, bb, 1 : 1 + H, 1 : 1 + W],
                        x_src[:, bb],
                    )

            out_v = out.ap().rearrange(
                "b (t p) h w -> t p b (h w)", t=NT
            )  # p = 128 partition-channels per tile

            for nt in range(NT):
                acc = apool.tile([128, FREE], dt.float32, tag="acc")
                acc_v = acc[:].rearrange("p (b h w) -> p b h w", b=B_LOC, h=H, w=W)
                dkt = []
                for k in range(9):
                    a, bcol = offs[k]
                    xwin = xpad_v[:, :, a : a + H, bcol : bcol + W]
                    scal = nbneg[:, nt * 9 + k : nt * 9 + k + 1]
                    d = dpool.tile([128, FREE], dt.float32, tag="d")
                    d_v = d[:].rearrange("p (b h w) -> p b h w", b=B_LOC, h=H, w=W)
                    # d = |x_win - nb|  (Abs activation, free per-partition bias)
                    nc.scalar.activation(d_v, xwin, AF.Abs, bias=scal, scale=1.0)
                    dkt.append(d)
                    if k == 1:
                        nc.vector.tensor_tensor(
                            acc[:], dkt[0][:], dkt[1][:], Alu.max
                        )
                    elif k > 1:
                        nc.vector.tensor_tensor(acc[:], acc[:], d[:], Alu.max)
                nc.sync.dma_start(out_v[nt], acc[:].rearrange("p (b s) -> p b s", b=B_LOC))

    nc.compile()
    return nc


def _get_module():
    if "nc" not in _module_cache:
        _module_cache["nc"] = _build_module()
    return _module_cache["nc"]


def _run(x, neighbors, trace=False):
    from concourse import bass_utils

    x = np.ascontiguousarray(x, dtype=np.float32)
    neighbors = np.ascontiguousarray(neighbors, dtype=np.float32)
    in_maps = []
    for core in range(NCORES):
        bg, ng = divmod(core, NG)
        in_maps.append(
            {
                "x": x[bg * B_LOC : (bg + 1) * B_LOC],
                "neighbors": neighbors[ng * N_LOC : (ng + 1) * N_LOC],
            }
        )
    res = bass_utils.run_bass_kernel_spmd(
        _get_module(), in_maps, core_ids=list(range(NCORES)), trace=trace
    )
    out = np.empty((B, NUM * C, H, W), dtype=np.float32)
    for core in range(NCORES):
        bg, ng = divmod(core, NG)
        out[bg * B_LOC : (bg + 1) * B_LOC, ng * N_LOC * C : (ng + 1) * N_LOC * C] = (
            res.results[core]["out"]
        )
    return out, res


def kernel(x, neighbors):
    out, _ = _run(x, neighbors, trace=False)
    return out


# revision 7
# speedup vs baseline: 1.3241x; 1.3241x over previous
"""ConvNearestNeightbor Trainium2 kernel.

out[b, n*C+c, i, j] = max_k |x[b,c,i-r_k,j-c_k] - neighbors[n,c,k]|
over the 9 zero-padded 3x3 shifts (r_k, c_k).

Sharding: 8 cores = 4 batch-groups x 2 num-groups.
Per core: B_loc=4 batches, N_loc=16 codebook entries.
Partition layout: (nn in 0..3, c in 0..31) -> 128 partitions, with the
codebook tile index nt in 0..3 selecting n = nt*4+nn.
Free dim per op: (b in 0..3, 32x32 pixels) = 4096.

Per (nt): 9 abs-diff planes d_k = |x_win_k - nb_k| are produced
(mostly on ScalarE via Abs activation with per-partition bias -nb;
optionally a few on VectorE via tensor_scalar subtract + bitwise-and
sign clear), then folded with tensor_tensor max on VectorE.
PREC="fp16" keeps d/acc in fp16 (2x DVE fold rate, one extra rounding
of ~2^-11 relative); PREC="fp32" is bit-exact vs the fp32 reference.
"""

import numpy as np

B, C, H, W = 16, 32, 32, 32
NUM = 32
NCORES = 8
BG, NG = 4, 2          # batch groups x num groups
B_LOC = B // BG        # 4
N_LOC = NUM // NG      # 16
NT = N_LOC // 4        # 4 codebook tiles of 4 n each
PH, PW = H + 2, W + 2  # 34 x 34 padded image
FREE = B_LOC * H * W   # 4096

PREC = "fp16"          # "fp16" or "fp32"
# shifts produced on VectorE instead of ScalarE (must be 4B-aligned
# window offsets for fp16 4x mode: k in {0,2,3,5,6,8})
DVE_PROD_KS = (0, 2)

_module_cache = {}


def _build_module():
    import concourse.bacc as bacc
    import concourse.mybir as mybir
    import concourse.tile as tile

    dt = mybir.dt
    Alu = mybir.AluOpType
    AF = mybir.ActivationFunctionType

    cdt = dt.float16 if PREC == "fp16" else dt.float32
    idt = dt.uint16 if PREC == "fp16" else dt.uint32
    mask = 0x7FFF if PREC == "fp16" else 0x7FFFFFFF

    nc = bacc.Bacc("TRN2", debug=False)
    x = nc.dram_tensor("x", [B_LOC, C, H, W], dt.float32, kind="ExternalInput")
    nb = nc.dram_tensor("neighbors", [N_LOC, C, 9], dt.float32, kind="ExternalInput")
    out = nc.dram_tensor(
        "out", [B_LOC, N_LOC * C, H, W], dt.float32, kind="ExternalOutput"
    )

    # window start offsets within the padded 34x34 image for the 9 shifts
    # k = (row+1)*3 + (col+1), window starts at (1-row, 1-col)
    offs = []
    for row in (-1, 0, 1):
        for col in (-1, 0, 1):
            offs.append((1 - row, 1 - col))

    with tile.TileContext(nc) as tc:
        with (
            tc.tile_pool(name="const", bufs=1) as cpool,
            tc.tile_pool(name="accp", bufs=3) as apool,
            tc.tile_pool(name="dp", bufs=6) as dpool,
        ):
            nbt = cpool.tile([128, NT * 9], dt.float32, tag="nbt")
            # nbt[(nn,c), (t,k)] = neighbors[t*4+nn, c, k]
            nb_src = nb.ap().rearrange("(t nn) c k -> (nn c) t k", nn=4)
            nbt_v = nbt[:].rearrange("p (t k) -> p t k", t=NT)
            nc.sync.dma_start(nbt_v, nb_src)
            # negated neighbors: ACT bias computes Abs(x + (-nb))
            nbneg = cpool.tile([128, NT * 9], dt.float32, tag="nbneg")
            nc.scalar.mul(nbneg[:], nbt[:], -1.0)

            # raw x load (contiguous, fast descriptors), cast to compute dtype
            xraw = cpool.tile([128, B_LOC * H * W], cdt, tag="xraw")
            xraw_v = xraw[:].rearrange("p (b h w) -> p b h w", b=B_LOC, h=H, w=W)
            x_src = x.ap().rearrange("b c h w -> c b h w")
            for nn in range(4):
                dst = xraw_v[nn * 32 : (nn + 1) * 32]
                if PREC == "fp16":
                    nc.gpsimd.dma_start(
                        dst.rearrange("c b h w -> c b (h w)"),
                        x_src.rearrange("c b h w -> c b (h w)"),
                    )
                else:
                    nc.sync.dma_start(
                        dst.rearrange("c b h w -> c b (h w)"),
                        x_src.rearrange("c b h w -> c b (h w)"),
                    )

            # padded image, borders zero; interior copied on GpSimd
            xpad = cpool.tile([128, B_LOC * PH * PW], cdt, tag="xpad")
            nc.gpsimd.memset(xpad[:], 0.0)
            xpad_v = xpad[:].rearrange("p (b h w) -> p b h w", b=B_LOC, h=PH, w=PW)
            nc.gpsimd.tensor_copy(xpad_v[:, :, 1 : 1 + H, 1 : 1 + W], xraw_v)

            out_v = out.ap().rearrange(
                "b (t p) h w -> t p b (h w)", t=NT
            )  # p = 128 partition-channels per tile

            for nt in range(NT):
                acc = apool.tile([128, FREE], cdt, tag="acc")
                nfold = 0
                first = None  # first produced d tile, folded on second
                for k in range(9):
                    a, bcol = offs[k]
                    xwin = xpad_v[:, :, a : a + H, bcol : bcol + W]
                    d = dpool.tile([128, FREE], cdt, tag="d")
                    d_v = d[:].rearrange("p (b h w) -> p b h w", b=B_LOC, h=H, w=W)
                    if k in DVE_PROD_KS:
                        # d = x_win - nb ; then clear sign bit -> |d|
                        nc.vector.tensor_scalar(
                            d_v, xwin, nbt[:, nt * 9 + k : nt * 9 + k + 1], None,
                            Alu.subtract,
                        )
                        nc.vector.tensor_scalar(
                            d[:].bitcast(idt), d[:].bitcast(idt), mask, None,
                            Alu.bitwise_and,
                        )
                    else:
                        # d = |x_win + (-nb)| on ScalarE
                        nc.scalar.activation(
                            d_v, xwin, AF.Abs,
                            bias=nbneg[:, nt * 9 + k : nt * 9 + k + 1], scale=1.0,
                        )
                    if first is None:
                        first = d
                    elif nfold == 0:
                        nc.vector.tensor_tensor(acc[:], first[:], d[:], Alu.max)
                        nfold = 1
                    else:
                        nc.vector.tensor_tensor(acc[:], acc[:], d[:], Alu.max)
                acc_s = acc[:].rearrange("p (b s) -> p b s", b=B_LOC)
                if PREC == "fp16":
                    nc.gpsimd.dma_start(out_v[nt], acc_s)  # SWDGE cast fp16->f32
                else:
                    nc.sync.dma_start(out_v[nt], acc_s)

    nc.compile()
    return nc


def _get_module():
    if "nc" not in _module_cache:
        _module_cache["nc"] = _build_module()
    return _module_cache["nc"]


def _run(x, neighbors, trace=False):
    from concourse import bass_utils

    x = np.ascontiguousarray(x, dtype=np.float32)
    neighbors = np.ascontiguousarray(neighbors, dtype=np.float32)
    in_maps = []
    for core in range(NCORES):
        bg, ng = divmod(core, NG)
        in_maps.append(
            {
                "x": x[bg * B_LOC : (bg + 1) * B_LOC],
                "neighbors": neighbors[ng * N_LOC : (ng + 1) * N_LOC],
            }
        )
    res = bass_utils.run_bass_kernel_spmd(
        _get_module(), in_maps, core_ids=list(range(NCORES)), trace=trace
    )
    out = np.empty((B, NUM * C, H, W), dtype=np.float32)
    for core in range(NCORES):
        bg, ng = divmod(core, NG)
        out[bg * B_LOC : (bg + 1) * B_LOC, ng * N_LOC * C : (ng + 1) * N_LOC * C] = (
            res.results[core]["out"]
        )
    return out, res


def kernel(x, neighbors):
    out, _ = _run(x, neighbors, trace=False)
    return out
